# revision 34
# baseline (speedup 1.0000x reference)
"""Attention kernel for Trainium2, SPMD across 8 NeuronCores.

Problem: x[4, 4096, 512]; Q,K,V = x@W* + b* (d_head=64);
Z = softmax(Q K^T / 8) V  -> [4, 4096, 64]

Sharding: data-parallel over batch (4) x query-halves (2) = 8 cores.
Each core handles 2048 queries of one batch against all 4096 keys of
that batch.  The key/value rows are fed in rolled order so every core's
queries sit at rows 0..2047 of its input -- softmax(QK^T)V is invariant
to a permutation of the key axis, so the result is exact.

Device algorithm (per core), bf16 matmuls with f32 PSUM accumulation:
  - x^T arrives pre-transposed [512, 4096] (host layout prep), loaded in
    [128, 1024] pieces split across both HWDGE engines, cast to bf16 on
    DVE/Pool; weights come in via gpsimd casting DMAs
  - streamed per 1024-col stripe: Q^T projection (stripes 0-1), fused
    [V^T; K^T] projection (chains interleaved pairwise so accumulating
    matmuls alternate PSUM banks), V^T -> V-natural PE transposes
    (+ones column), then the flash sweep for query chunks 0-1 over that
    stripe's key blocks -- PE/ACT start ~15us in while later stripes load
  - scores computed TRANSPOSED: score^T[k, q] blocks, lhsT=K^T-block
    (contraction=64); even/odd key blocks row-packed onto partition
    groups 0-63 / 64-127 so pairs run concurrently
  - exp on ScalarE straight out of PSUM ([128, 2, 512] groups, 1/8 fused)
  - P^T @ [V|1] accumulates Z^T[64, q] AND the softmax denominator
    (row 64) across all 32 key blocks
  - query chunks 2-3 sweep after the stripes; division tails
    (reciprocal + rank-1 broadcast matmul + multiply) software-pipelined
  - output is Z^T [64, 2048] f32; the host transposes back.
"""

import os
import sys

import numpy as np

for _p in ("/opt/trn_rl_repo", "/root/.axon_site/_ro/trn_rl_repo"):
    if os.path.isdir(_p) and _p not in sys.path:
        sys.path.insert(0, _p)

import concourse.bass as bass
import concourse.mybir as mybir
from concourse import bacc
from concourse.bass_utils import run_bass_kernel_spmd
from concourse.masks import make_identity
from concourse.tile import TileContext

F32 = mybir.dt.float32
BF16 = mybir.dt.bfloat16

B = 4          # batch
S = 4096       # sequence (keys)
SQ = 2048      # queries per core
W = 512        # d_model
E = 64         # d_head
P = 128
WC = W // P    # 4 w-chunks
NQC = SQ // 512  # 4 query chunks of 512
NKB = S // P   # 32 key blocks of 128
G = 2          # key blocks per exp group

N_CORES = 8


def build_graph() -> bass.Bass:
    nc = bacc.Bacc(
        "TRN2",
        target_bir_lowering=False,
        debug=False,
        num_devices=N_CORES,
        enable_partition_id=False,
        num_swdge_queues=1,
    )

    xt_d = nc.declare_dram_parameter("xt", [W, S], F32, isOutput=False)
    wq_d = nc.declare_dram_parameter("wq", [W, E], F32, isOutput=False)
    # wvk packs [Wv | Wk] -> [512, 128]
    wvk_d = nc.declare_dram_parameter("wvk", [W, 2 * E], F32, isOutput=False)
    bq_d = nc.declare_dram_parameter("bq", [E], F32, isOutput=False)
    # bkv packs [bv; bk] -> [128]
    bkv_d = nc.declare_dram_parameter("bkv", [2 * E], F32, isOutput=False)
    out_d = nc.declare_dram_parameter("out", [E, SQ], F32, isOutput=True)

    xt_view = xt_d.rearrange("(c p) s -> c p s", p=P)

    with TileContext(nc) as tc:
        with (
            tc.tile_pool(name="consts", bufs=1) as consts,
            tc.tile_pool(name="persist", bufs=1) as persist,
            tc.tile_pool(name="stage", bufs=2) as stage,
            # PSUM (8 banks): pa-tag 2x[128,512] = 2 (proj chains +
            # V-transposes), sp-tag 2x[128,2,512] = 4 (score groups),
            # zp-tag 2x[65,512] = 2 (Z^T accumulators / bcast tiles)
            tc.tile_pool(name="pa", bufs=2, space="PSUM") as paP,
            tc.tile_pool(name="sp", bufs=2, space="PSUM") as spP,
            tc.tile_pool(name="zp", bufs=2, space="PSUM") as zpP,
            tc.tile_pool(name="pexp", bufs=4) as peP,
            tc.tile_pool(name="fin", bufs=2) as finP,
        ):
            # --- constants ---
            id64 = consts.tile([E, E], BF16)
            make_identity(nc, id64)
            onesw = consts.tile([E + 1, E], F32)
            nc.gpsimd.memset(onesw[E : E + 1, :], 1.0)
            bq_t = consts.tile([E, 1], F32)
            nc.sync.dma_start(bq_t, bq_d[:, None])
            bkv_t = consts.tile([P, 1], F32)
            nc.sync.dma_start(bkv_t, bkv_d[:, None])
            wq_b = consts.tile([P, WC, E], BF16)
            nc.gpsimd.dma_start(wq_b, wq_d.rearrange("(c p) e -> p c e", p=P))
            wvk_b = consts.tile([P, WC, 2 * E], BF16)
            nc.gpsimd.dma_start(wvk_b, wvk_d.rearrange("(c p) e -> p c e", p=P))

            # --- persistent activations ---
            xtb = persist.tile([P, WC, S], BF16)      # x^T bf16
            qt = persist.tile([P, SQ], BF16)          # Q^T on both halves
            kvt = persist.tile([P, S], BF16)          # 0:64 V^T, 64:128 K^T
            ktd = persist.tile([P, S], BF16)          # 0:64 K^T (copy)
            vnat = persist.tile([P, NKB, E + 1], BF16)  # V natural + ones
            nc.gpsimd.memset(vnat[:, :, E : E + 1], 1.0)

            zps = {}

            def proj_pair(chunks):
                """Interleaved 4-matmul projection chains, each into its
                own pa-pool slot (bank-alternating so the accumulating
                matmuls overlap).  chunk = (kind, cs)."""
                tiles = []
                for kind, cs in chunks:
                    t = paP.tile([P, 512], F32, tag="pa", name=f"pj{kind}")
                    tiles.append(t)
                for wc in range(WC):
                    for (kind, cs), pt in zip(chunks, tiles):
                        wgt = wq_b if kind == "q" else wvk_b
                        mh = E if kind == "q" else P
                        nc.tensor.matmul(
                            pt[0:mh, :], wgt[:, wc, :], xtb[:, wc, cs],
                            start=(wc == 0), stop=(wc == WC - 1),
                        )
                for (kind, cs), pt in zip(chunks, tiles):
                    if kind == "q":
                        nc.vector.tensor_scalar_add(
                            qt[0:E, cs], pt[0:E, :], bq_t
                        )
                    else:
                        nc.vector.tensor_scalar_add(kvt[:, cs], pt, bkv_t)

            def sweep_pair(qca, qcb, g0, g1):
                """Score+exp+PV for TWO query chunks over exp-groups
                [g0, g1), interleaved so consecutive PV matmuls alternate
                between the two accumulators' PSUM banks (overlapping
                instead of serializing on one bank's read-modify-write)."""
                for qc in (qca, qcb):
                    if qc not in zps:
                        zps[qc] = zpP.tile(
                            [E + 1, 512], F32, tag="zp", name=f"zpacc{qc}"
                        )
                for g in range(g0, g1):
                    kbs = list(range(g * G, min((g + 1) * G, NKB)))
                    n = len(kbs)
                    sps, pes = {}, {}
                    for qc in (qca, qcb):
                        qs = slice(qc * 512, (qc + 1) * 512)
                        sp = spP.tile(
                            [P, G, 512], F32, tag="sp", name=f"sp{qc % 2}"
                        )
                        for j, kb in enumerate(kbs):
                            if kb % 2 == 0:
                                lhs = ktd[0:E, kb * P : (kb + 1) * P]
                                rhs = qt[0:E, qs]
                            else:
                                lhs = kvt[E:P, kb * P : (kb + 1) * P]
                                rhs = qt[E:P, qs]
                            nc.tensor.matmul(
                                sp[:, j, :], lhs, rhs, start=True, stop=True
                            )
                        pe = peP.tile(
                            [P, G, 512], BF16, tag="pe", name=f"pe{qc % 2}"
                        )
                        nc.scalar.activation(
                            pe[:, :n, :], sp[:, :n, :],
                            mybir.ActivationFunctionType.Exp, scale=0.125,
                        )
                        sps[qc], pes[qc] = sp, pe
                    for j, kb in enumerate(kbs):
                        for qc in (qca, qcb):
                            nc.tensor.matmul(
                                zps[qc], vnat[:, kb, :], pes[qc][:, j, :],
                                start=(kb == 0), stop=(kb == NKB - 1),
                            )

            def sweep_one(qc, g0, g1):
                """Score+exp+PV for a single query chunk."""
                qs = slice(qc * 512, (qc + 1) * 512)
                zp = zps[qc]
                for g in range(g0, g1):
                    kbs = list(range(g * G, min((g + 1) * G, NKB)))
                    n = len(kbs)
                    sp = spP.tile([P, G, 512], F32, tag="sp", name="spo")
                    for j, kb in enumerate(kbs):
                        if kb % 2 == 0:
                            lhs = ktd[0:E, kb * P : (kb + 1) * P]
                            rhs = qt[0:E, qs]
                        else:
                            lhs = kvt[E:P, kb * P : (kb + 1) * P]
                            rhs = qt[E:P, qs]
                        nc.tensor.matmul(
                            sp[:, j, :], lhs, rhs, start=True, stop=True
                        )
                    pe = peP.tile([P, G, 512], BF16, tag="pe", name="peo")
                    nc.scalar.activation(
                        pe[:, :n, :], sp[:, :n, :],
                        mybir.ActivationFunctionType.Exp, scale=0.125,
                    )
                    for j, kb in enumerate(kbs):
                        nc.tensor.matmul(
                            zp, vnat[:, kb, :], pe[:, j, :],
                            start=(kb == 0), stop=(kb == NKB - 1),
                        )

            def finish_sweep(qc):
                # pull Z^T+denom out of PSUM right away to free the slot
                zsb = finP.tile([E + 1, 512], F32, tag="zsb")
                nc.vector.tensor_copy(zsb, zps[qc])
                del zps[qc]
                return zsb

            def tail(qc, zsb):
                qs = slice(qc * 512, (qc + 1) * 512)
                rdt = finP.tile([E + 1, 512], F32, tag="rdt")
                nc.vector.reciprocal(rdt[E : E + 1, :], zsb[E : E + 1, :])
                bc = paP.tile([E + 1, 512], F32, tag="pa", name=f"bc{qc}")
                nc.tensor.matmul(
                    bc[0:E, :], onesw[E : E + 1, :], rdt[E : E + 1, :],
                    start=True, stop=True,
                )
                bcs = finP.tile([E, 512], F32, tag="bcs")
                nc.vector.tensor_copy(bcs, bc[0:E, :])
                zf = finP.tile([E, 512], F32, tag="zf")
                nc.vector.tensor_tensor(
                    zf, zsb[0:E, :], bcs, mybir.AluOpType.mult
                )
                nc.gpsimd.dma_start(out_d[:, qs], zf)

            # --- streamed stripes (1024 cols each) ---
            for qq in range(4):
                qsl = slice(qq * 1024, (qq + 1) * 1024)
                for half in range(2):
                    hsl = slice(
                        qq * 1024 + half * 512, qq * 1024 + half * 512 + 512
                    )
                    for wc in range(WC):
                        xf = stage.tile([P, 512], F32, tag=f"xf{wc}_{half}")
                        dma_eng = nc.sync if wc % 2 == 0 else nc.scalar
                        dma_eng.dma_start(xf, xt_view[wc, :, hsl])
                        cast_eng = nc.gpsimd if wc == 3 else nc.vector
                        cast_eng.tensor_copy(xtb[:, wc, hsl], xf)

                # projections, chains interleaved pairwise
                c0 = slice(qq * 1024, qq * 1024 + 512)
                c1 = slice(qq * 1024 + 512, qq * 1024 + 1024)
                if qq < 2:
                    proj_pair([("q", c0), ("kv", c0)])
                    proj_pair([("q", c1), ("kv", c1)])
                    nc.gpsimd.dma_start(qt[E:P, qsl], qt[0:E, qsl])
                else:
                    proj_pair([("kv", c0), ("kv", c1)])
                nc.gpsimd.dma_start(ktd[0:E, qsl], kvt[E:P, qsl])

                # V natural (+ones col) via PE transpose
                for kb in range(qq * 8, qq * 8 + 8):
                    vps = paP.tile([P, E], BF16, tag="pa", name="vps")
                    nc.tensor.transpose(
                        vps, kvt[0:E, kb * P : (kb + 1) * P], id64
                    )
                    nc.vector.tensor_copy(vnat[:, kb, 0:E], vps)

                # sweep query chunks 0-1 over this stripe's key blocks
                gpq = 8 // G
                sweep_pair(0, 1, qq * gpq, (qq + 1) * gpq)

            zsb0 = finish_sweep(0)
            zsb1 = finish_sweep(1)

            # --- back half: query chunks 2-3 (all data resident) ---
            NG = NKB // G
            sweep_pair(2, 3, 0, NG // 2)
            tail(0, zsb0)
            tail(1, zsb1)
            sweep_pair(2, 3, NG // 2, NG - 3)
            # stagger the endings so qc2's tail overlaps qc3's last groups
            sweep_one(2, NG - 3, NG)
            zsb2 = finish_sweep(2)
            tail(2, zsb2)
            sweep_one(3, NG - 3, NG)
            zsb3 = finish_sweep(3)
            tail(3, zsb3)

    nc.compile()
    return nc


_GRAPH_CACHE: bass.Bass | None = None


def _get_graph() -> bass.Bass:
    global _GRAPH_CACHE
    if _GRAPH_CACHE is None:
        _GRAPH_CACHE = build_graph()
    return _GRAPH_CACHE


def _make_in_maps(x, Wq, bq, Wk, bk, Wv, bv):
    x = np.asarray(x, dtype=np.float32)
    wq = np.ascontiguousarray(np.asarray(Wq, dtype=np.float32))
    wvk = np.ascontiguousarray(
        np.concatenate(
            [np.asarray(Wv, dtype=np.float32), np.asarray(Wk, dtype=np.float32)],
            axis=1,
        )
    )
    bq_ = np.ascontiguousarray(np.asarray(bq, dtype=np.float32))
    bkv = np.ascontiguousarray(
        np.concatenate(
            [np.asarray(bv, dtype=np.float32), np.asarray(bk, dtype=np.float32)]
        )
    )
    in_maps = []
    for c in range(N_CORES):
        b, h = divmod(c, 2)
        xl = np.roll(x[b], -h * SQ, axis=0)
        xt = np.ascontiguousarray(xl.T)
        in_maps.append({"xt": xt, "wq": wq, "wvk": wvk, "bq": bq_, "bkv": bkv})
    return in_maps


def _run(inputs: dict, trace: bool = False):
    nc = _get_graph()
    in_maps = _make_in_maps(**inputs)
    res = run_bass_kernel_spmd(
        nc, in_maps, core_ids=list(range(N_CORES)), trace=trace
    )
    out = np.zeros((B, S, E), dtype=np.float32)
    for c in range(N_CORES):
        b, h = divmod(c, 2)
        out[b, h * SQ : (h + 1) * SQ, :] = res.results[c]["out"].T
    return out, res


def kernel(**inputs) -> np.ndarray:
    out, _ = _run(inputs, trace=False)
    return out
